# revision 40
# baseline (speedup 1.0000x reference)
"""Triangle attention (starting node) Bass kernel for 8 trn2 NeuronCores.

Math (B=1, N=256, D=128, H=4, E=32):
  bias[h,j,k] = sum_d P[j,k,d] Wb[d,h]
  q[h,i,j,e]  = sum_d P[i,j,d] Wq[d,h*E+e]   (k,v analogous)
  S[i,h,j,k]  = (q . k) * E**-0.5 + bias[h,j,k]
  out[i,j,:]  = (softmax_k S @ v) merged over h, @ Wo

Sharding: rows i are split across 8 cores (32 rows each). The bias couples all
rows, so each core recomputes the FULL bias from the full (replicated,
device-resident) pair tensor — one program, one dispatch, no host gather
between passes.

All PE operands are bf16 (1 cycle/row vs fp32's 4 on the trn2 PE; halves HBM
traffic); PSUM accumulation, exp, and the softmax normalization stay fp32.
Tolerance is 2e-2 fro rel err; bf16 lands ~3e-3.

On-chip layout is "T-form": scores are built transposed, ST[k, j] per head, so
softmax normalization sums over the partition axis (done on the PE with a ones
matmul, replicated x32 for free) and the AV matmul consumes ST directly with
no transpose of the attention matrix. The host supplies pairwise_repr already
transposed to [d, i*N+token] so every on-chip matmul operand has its
contraction axis on partitions.
"""

import os
from contextlib import ExitStack

import numpy as np

N = 256
D = 128
H = 4
E = 32
NCORES = 8
RPC = N // NCORES  # rows per core
SCALE = float(E) ** -0.5

_cache = {}

_legal_ctr = [0]


def _legalize_waits(nc):
    """Walrus caps semaphore wait-commands per lowered instruction (LDWEIGHTS
    holds only one). Hoist excess waits of every instruction into fresh
    single-wait NoOps on the same engine, inserted right before it — same
    wait point, so timing/deadlock semantics are unchanged."""
    import bass_rust

    for fn in nc.m.functions:
        for blk in fn.blocks:
            ins = blk.instructions
            i = 0
            while i < len(ins):
                inst = ins[i]
                si = inst.sync_info
                if si is None or inst.engine is None:
                    i += 1
                    continue
                waits = si.on_wait
                if len(waits) <= 1:
                    i += 1
                    continue
                for w in waits[:-1]:
                    _legal_ctr[0] += 1
                    n = bass_rust.InstNoOp(name=f"I-lgl-{_legal_ctr[0]}")
                    n.engine = inst.engine
                    n.sync_info = bass_rust.SyncInfo(on_wait=[w], on_update=[])
                    ins.insert(i, n)
                    i += 1
                si.on_wait = [waits[-1]]
                inst.sync_info = si
                i += 1


def _build_fused():
    """Single program: sharded bias + AllGather + row-shard attention, bf16.

    Inputs per core (bf16):
      xT   [D, RPC*N]  own row shard, transposed:  xT[d, il*N + t] = P[i, t, d]
      wb [D,H], wq/wk/wv [D,D] (wq pre-scaled by E**-0.5 on host), wo [D,D]
    Output: outT [RPC, D, N] fp32 with outT[il, d, j] = out[i, j, d].

    Each core computes bias columns for its own 32 rows j from xt_sb, then an
    HBM-HBM AllGather (flat core-major concat) distributes the full [h,j,k]
    bias to every core.
    """
    import concourse.bass as bass
    import concourse.mybir as mybir
    import concourse.tile as tile
    from concourse.masks import make_identity

    f32 = mybir.dt.float32
    bf16 = mybir.dt.bfloat16
    AF = mybir.ActivationFunctionType
    nc = bass.Bass("TRN2", target_bir_lowering=False, debug=False,
                   enable_asserts=False, num_devices=NCORES)
    xT = nc.dram_tensor("xT", [D, RPC * N], bf16, kind="ExternalInput").ap()
    wb = nc.dram_tensor("wb", [D, H], bf16, kind="ExternalInput").ap()
    wq = nc.dram_tensor("wq", [D, D], bf16, kind="ExternalInput").ap()
    wk = nc.dram_tensor("wk", [D, D], bf16, kind="ExternalInput").ap()
    wv = nc.dram_tensor("wv", [D, D], bf16, kind="ExternalInput").ap()
    wo = nc.dram_tensor("wo", [D, D], bf16, kind="ExternalInput").ap()
    outT = nc.dram_tensor("outT", [RPC, D, N], f32, kind="ExternalOutput").ap()

    RB = 8  # rows per projection batch
    with ExitStack() as ctx:
        tc = ctx.enter_context(tile.TileContext(nc))
        singles = ctx.enter_context(tc.tile_pool(name="singles", bufs=1))
        dram = ctx.enter_context(tc.tile_pool(name="dram", bufs=2, space="DRAM"))
        qk_pool = ctx.enter_context(tc.tile_pool(name="qk", bufs=4))
        v_pool = ctx.enter_context(tc.tile_pool(name="v", bufs=3))
        es_pool = ctx.enter_context(tc.tile_pool(name="es", bufs=4))
        sm_pool = ctx.enter_context(tc.tile_pool(name="sm", bufs=3))
        out_pool = ctx.enter_context(tc.tile_pool(name="outp", bufs=3))
        s_psum = ctx.enter_context(tc.tile_pool(name="spsum", bufs=2, space="PSUM"))
        o_psum = ctx.enter_context(tc.tile_pool(name="opsum", bufs=2, space="PSUM"))
        m_psum = ctx.enter_context(tc.tile_pool(name="mpsum", bufs=2, space="PSUM"))

        wb_sb = singles.tile([D, H], bf16)
        wq_sb = singles.tile([D, D], bf16)
        wk_sb = singles.tile([D, D], bf16)
        wv_sb = singles.tile([D, D], bf16)
        wo_sb = singles.tile([D, D], bf16)
        identf = singles.tile([128, 128], f32)
        ident = singles.tile([128, 128], bf16)
        ones = singles.tile([128, E], bf16)
        bias_sb = singles.tile([128, 2 * H * N], bf16)  # [k, kh*1024 + h*256 + j]
        xt_sb = singles.tile([D, RPC * N], bf16)
        qTall = singles.tile([128, RPC * N], bf16)  # [he, il*N + j]
        kTall = singles.tile([128, RPC * N], bf16)  # [he, il*N + k]
        vall = singles.tile([128, RPC * N], bf16)   # [ktok, il*N + half*128 + he]

        nc.sync.dma_start(out=wb_sb, in_=wb)
        for c in range(2):
            sl = slice(c * RPC * N // 2, (c + 1) * RPC * N // 2)
            nc.sync.dma_start(out=xt_sb[:, sl], in_=xT[:, sl])
        nc.sync.dma_start(out=wq_sb, in_=wq)
        nc.sync.dma_start(out=wk_sb, in_=wk)
        nc.sync.dma_start(out=wv_sb, in_=wv)
        nc.sync.dma_start(out=wo_sb, in_=wo)
        make_identity(nc, identf)
        nc.vector.tensor_copy(ident, identf)
        nc.vector.memset(ones, 1.0)

        # --- bias shard: st[k, kh*H*RPC + h*RPC + jl] for own rows jl, then
        # AllGather (flat core-major concat) -> full bias_sb[k, kh*1024+h*256+j]
        # with j = c*RPC + jl.
        pb = s_psum.tile([128, 2 * H * RPC], f32, tag="s")  # [k, kh*128 + jl*4 + h]
        for jl in range(RPC):
            for kh in range(2):
                nc.tensor.matmul(
                    pb[:, kh * 128 + jl * 4: kh * 128 + jl * 4 + 4],
                    xt_sb[:, jl * N + kh * 128: jl * N + kh * 128 + 128],
                    wb_sb,
                    start=True, stop=True,
                )
        st = sm_pool.tile([128, 2 * H * RPC], bf16, tag="rs")
        for kh in range(2):
            # st[:, kh*128 + h*RPC + jl] = pb[:, kh*128 + jl*4 + h]
            nc.vector.tensor_copy(
                st[:, kh * H * RPC:(kh + 1) * H * RPC].rearrange(
                    "p (h j) -> p h j", h=H),
                pb[:, kh * H * RPC:(kh + 1) * H * RPC].rearrange(
                    "p (j h) -> p h j", h=H),
            )
        bshard = dram.tile([128, 2 * H * RPC], bf16)
        bgather = dram.tile([NCORES, 128, 2 * H * RPC], bf16)
        nc.gpsimd.dma_start(out=bshard[:], in_=st)
        nc.gpsimd.collective_compute(
            "AllGather", mybir.AluOpType.bypass,
            replica_groups=[list(range(NCORES))],
            ins=[bshard.opt()], outs=[bgather.opt()])

        # --- projections + v for ALL rows (overlap the collective)
        for wsb, dst in ((wq_sb, qTall), (wk_sb, kTall)):
            for c in range(RPC * N // 512):
                pp = m_psum.tile([128, 512], f32, tag="m")
                nc.tensor.matmul(
                    pp, wsb, xt_sb[:, c * 512:(c + 1) * 512],
                    start=True, stop=True)
                nc.vector.tensor_copy(dst[:, c * 512:(c + 1) * 512], pp)
        for r in range(RPC):
            roff = r * N
            pv = m_psum.tile([128, 512], f32, tag="m")
            for half in range(2):
                nc.tensor.matmul(
                    pv[:, half * 128:(half + 1) * 128],
                    xt_sb[:, roff + half * 128: roff + half * 128 + 128],
                    wv_sb,
                    start=True, stop=True)
            nc.vector.tensor_copy(vall[:, roff: roff + N], pv[:, 0:N])

        bias_v = bias_sb.rearrange("p (kh h j) -> p kh h j", kh=2, h=H)
        for c in range(NCORES):
            # bias_v[:, kh, h, c*RPC + jl] = bgather[c, :, kh*128 + h*RPC + jl]
            nc.sync.dma_start(
                out=bias_v[:, :, :, c * RPC:(c + 1) * RPC],
                in_=bgather[c].rearrange("p (kh h j) -> p kh h j", kh=2, h=H))

        for r in range(RPC):
            roff = r * N
            est = []
            for kh in range(2):
                sp = s_psum.tile([128, H * N], f32, tag="s")
                for h in range(H):
                    nc.tensor.matmul(
                        sp[:, h * N:(h + 1) * N],
                        ident,
                        bias_sb[:, kh * H * N + h * N: kh * H * N + (h + 1) * N],
                        start=True, stop=False)
                    nc.tensor.matmul(
                        sp[:, h * N:(h + 1) * N],
                        kTall[32 * h:32 * h + 32, roff + kh * 128: roff + kh * 128 + 128],
                        qTall[32 * h:32 * h + 32, roff: roff + N],
                        start=False, stop=True,
                        tile_position=(32 * h, 0))
                es = es_pool.tile([128, H * N], bf16, tag="es")
                nc.scalar.activation(es, sp, AF.Exp)
                est.append(es)

            # --- rowsums (replicated x32 via ones[128,E]) and AV
            po = o_psum.tile([128, 512], f32, tag="o")
            for h in range(H):
                for kh in range(2):
                    nc.tensor.matmul(
                        po[32 * h:32 * h + 32, 256:512],
                        ones,
                        est[kh][:, h * N:(h + 1) * N],
                        start=(kh == 0), stop=(kh == 1),
                        tile_position=(0, 32 * h))
            for h in range(H):
                for kh in range(2):
                    nc.tensor.matmul(
                        po[32 * h:32 * h + 32, 0:256],
                        vall[:, roff + kh * 128 + 32 * h: roff + kh * 128 + 32 * h + 32],
                        est[kh][:, h * N:(h + 1) * N],
                        start=(kh == 0), stop=(kh == 1),
                        tile_position=(0, 32 * h))

            rs_rec = sm_pool.tile([128, N], f32, tag="rs")
            nc.vector.reciprocal(rs_rec, po[:, 256:512])
            oT_sb = sm_pool.tile([128, N], bf16, tag="oT")
            nc.vector.tensor_mul(oT_sb, po[:, 0:256], rs_rec)

            # --- output projection: outT[d, j] = sum_he Wo[he,d] oT[he,j]
            pf = m_psum.tile([128, 512], f32, tag="m")
            nc.tensor.matmul(pf[:, 0:N], wo_sb, oT_sb, start=True, stop=True)
            o_sb = out_pool.tile([128, N], f32, tag="osb")
            nc.vector.tensor_copy(o_sb, pf[:, 0:N])
            nc.sync.dma_start(out=outT[r], in_=o_sb)
    return nc


def _get_programs():
    if "nc" not in _cache:
        _cache["nc"] = _build_fused()
        _legalize_waits(_cache["nc"])
    return _cache["nc"]


def kernel(pairwise_repr, mask, Wb, Wq, Wk, Wv, Wo):
    import ml_dtypes
    from concourse.bass_utils import run_bass_kernel_spmd

    bf = ml_dtypes.bfloat16
    nc = _get_programs()

    x = np.asarray(pairwise_repr, dtype=np.float32)[0]
    # xT[d, i*N + t] = x[i, t, d]
    xT = np.ascontiguousarray(x.reshape(N * N, D).T.astype(bf))
    shards = [np.ascontiguousarray(xT[:, c * RPC * N:(c + 1) * RPC * N])
              for c in range(NCORES)]
    wb = np.asarray(Wb, np.float32).astype(bf)
    # fold the attention scale into Wq so q comes out of the PE pre-scaled
    wq = (np.asarray(Wq, np.float32) * SCALE).astype(bf)
    wk = np.asarray(Wk, np.float32).astype(bf)
    wv = np.asarray(Wv, np.float32).astype(bf)
    wo = np.asarray(Wo, np.float32).astype(bf)

    core_ids = list(range(NCORES))
    in_maps = [{"xT": shards[c], "wb": wb, "wq": wq, "wk": wk,
                "wv": wv, "wo": wo} for c in range(NCORES)]
    kernel._last_in = in_maps
    res = run_bass_kernel_spmd(nc, in_maps, core_ids=core_ids, trace=False)

    kernel._last = res
    # outT [RPC, D, N] per core -> out[0, 32c+r, j, d] = outT_c[r, d, j]
    o = np.stack([res.results[c]["outT"] for c in range(NCORES)])
    out = o.transpose(0, 1, 3, 2).reshape(1, N, N, D)
    return np.ascontiguousarray(out.astype(np.float32))


# revision 41
# speedup vs baseline: 1.0195x; 1.0195x over previous
"""Triangle attention (starting node) Bass kernel for 8 trn2 NeuronCores.

Math (B=1, N=256, D=128, H=4, E=32):
  bias[h,j,k] = sum_d P[j,k,d] Wb[d,h]
  q[h,i,j,e]  = sum_d P[i,j,d] Wq[d,h*E+e]   (k,v analogous)
  S[i,h,j,k]  = (q . k) * E**-0.5 + bias[h,j,k]
  out[i,j,:]  = (softmax_k S @ v) merged over h, @ Wo

Sharding: rows i are split across 8 cores (32 rows each). The bias couples all
rows, so each core recomputes the FULL bias from the full (replicated,
device-resident) pair tensor — one program, one dispatch, no host gather
between passes.

All PE operands are bf16 (1 cycle/row vs fp32's 4 on the trn2 PE; halves HBM
traffic); PSUM accumulation, exp, and the softmax normalization stay fp32.
Tolerance is 2e-2 fro rel err; bf16 lands ~3e-3.

On-chip layout is "T-form": scores are built transposed, ST[k, j] per head, so
softmax normalization sums over the partition axis (done on the PE with a ones
matmul, replicated x32 for free) and the AV matmul consumes ST directly with
no transpose of the attention matrix. The host supplies pairwise_repr already
transposed to [d, i*N+token] so every on-chip matmul operand has its
contraction axis on partitions.
"""

import os
from contextlib import ExitStack

import numpy as np

N = 256
D = 128
H = 4
E = 32
NCORES = 8
RPC = N // NCORES  # rows per core
SCALE = float(E) ** -0.5

_cache = {}

_legal_ctr = [0]


def _legalize_waits(nc):
    """Walrus caps semaphore wait-commands per lowered instruction (LDWEIGHTS
    holds only one). Hoist excess waits of every instruction into fresh
    single-wait NoOps on the same engine, inserted right before it — same
    wait point, so timing/deadlock semantics are unchanged."""
    import bass_rust

    for fn in nc.m.functions:
        for blk in fn.blocks:
            ins = blk.instructions
            i = 0
            while i < len(ins):
                inst = ins[i]
                si = inst.sync_info
                if si is None or inst.engine is None:
                    i += 1
                    continue
                waits = si.on_wait
                if len(waits) <= 1:
                    i += 1
                    continue
                for w in waits[:-1]:
                    _legal_ctr[0] += 1
                    n = bass_rust.InstNoOp(name=f"I-lgl-{_legal_ctr[0]}")
                    n.engine = inst.engine
                    n.sync_info = bass_rust.SyncInfo(on_wait=[w], on_update=[])
                    ins.insert(i, n)
                    i += 1
                si.on_wait = [waits[-1]]
                inst.sync_info = si
                i += 1


def _build_fused():
    """Single program: sharded bias + AllGather + row-shard attention, bf16.

    Inputs per core (bf16):
      xT   [D, RPC*N]  own row shard, transposed:  xT[d, il*N + t] = P[i, t, d]
      wb [D,H], wq/wk/wv [D,D] (wq pre-scaled by E**-0.5 on host), wo [D,D]
    Output: outT [RPC, D, N] fp32 with outT[il, d, j] = out[i, j, d].

    Each core computes bias columns for its own 32 rows j from xt_sb, then an
    HBM-HBM AllGather (flat core-major concat) distributes the full [h,j,k]
    bias to every core.
    """
    import concourse.bass as bass
    import concourse.mybir as mybir
    import concourse.tile as tile
    from concourse.masks import make_identity

    f32 = mybir.dt.float32
    bf16 = mybir.dt.bfloat16
    AF = mybir.ActivationFunctionType
    nc = bass.Bass("TRN2", target_bir_lowering=False, debug=False,
                   enable_asserts=False, num_devices=NCORES)
    xT = nc.dram_tensor("xT", [D, RPC * N], bf16, kind="ExternalInput").ap()
    wb = nc.dram_tensor("wb", [D, H], bf16, kind="ExternalInput").ap()
    wq = nc.dram_tensor("wq", [D, D], bf16, kind="ExternalInput").ap()
    wk = nc.dram_tensor("wk", [D, D], bf16, kind="ExternalInput").ap()
    wv = nc.dram_tensor("wv", [D, D], bf16, kind="ExternalInput").ap()
    wo = nc.dram_tensor("wo", [D, D], bf16, kind="ExternalInput").ap()
    outT = nc.dram_tensor("outT", [RPC, D, N], f32, kind="ExternalOutput").ap()

    RB = 8  # rows per projection batch
    with ExitStack() as ctx:
        tc = ctx.enter_context(tile.TileContext(nc))
        singles = ctx.enter_context(tc.tile_pool(name="singles", bufs=1))
        dram = ctx.enter_context(tc.tile_pool(name="dram", bufs=2, space="DRAM"))
        qk_pool = ctx.enter_context(tc.tile_pool(name="qk", bufs=4))
        v_pool = ctx.enter_context(tc.tile_pool(name="v", bufs=3))
        es_pool = ctx.enter_context(tc.tile_pool(name="es", bufs=8))
        sm_pool = ctx.enter_context(tc.tile_pool(name="sm", bufs=3))
        out_pool = ctx.enter_context(tc.tile_pool(name="outp", bufs=3))
        s_psum = ctx.enter_context(tc.tile_pool(name="spsum", bufs=4, space="PSUM"))
        o_psum = ctx.enter_context(tc.tile_pool(name="opsum", bufs=2, space="PSUM"))
        m_psum = ctx.enter_context(tc.tile_pool(name="mpsum", bufs=2, space="PSUM"))

        wb_sb = singles.tile([D, H], bf16)
        wq_sb = singles.tile([D, D], bf16)
        wk_sb = singles.tile([D, D], bf16)
        wv_sb = singles.tile([D, D], bf16)
        wo_sb = singles.tile([D, D], bf16)
        identf = singles.tile([128, 128], f32)
        ident = singles.tile([128, 128], bf16)
        ones = singles.tile([128, E], bf16)
        bias_sb = singles.tile([128, 2 * H * N], bf16)  # [k, kh*1024 + h*256 + j]
        xt_sb = singles.tile([D, RPC * N], bf16)
        qTall = singles.tile([128, RPC * N], bf16)  # [he, il*N + j]
        kTall = singles.tile([128, RPC * N], bf16)  # [he, il*N + k]
        vall = singles.tile([128, RPC * N], bf16)   # [ktok, il*N + half*128 + he]

        nc.sync.dma_start(out=wb_sb, in_=wb)
        for c in range(2):
            sl = slice(c * RPC * N // 2, (c + 1) * RPC * N // 2)
            nc.sync.dma_start(out=xt_sb[:, sl], in_=xT[:, sl])
        nc.sync.dma_start(out=wq_sb, in_=wq)
        nc.sync.dma_start(out=wk_sb, in_=wk)
        nc.sync.dma_start(out=wv_sb, in_=wv)
        nc.sync.dma_start(out=wo_sb, in_=wo)
        make_identity(nc, identf)
        nc.vector.tensor_copy(ident, identf)
        nc.vector.memset(ones, 1.0)

        # --- bias shard: st[k, kh*H*RPC + h*RPC + jl] for own rows jl, then
        # AllGather (flat core-major concat) -> full bias_sb[k, kh*1024+h*256+j]
        # with j = c*RPC + jl.
        pb = s_psum.tile([128, 512], f32, tag="s")  # [k, kh*128 + jl*4 + h]
        for jl in range(RPC):
            for kh in range(2):
                nc.tensor.matmul(
                    pb[:, kh * 128 + jl * 4: kh * 128 + jl * 4 + 4],
                    xt_sb[:, jl * N + kh * 128: jl * N + kh * 128 + 128],
                    wb_sb,
                    start=True, stop=True,
                )
        st = sm_pool.tile([128, 2 * H * RPC], bf16, tag="rs")
        for kh in range(2):
            # st[:, kh*128 + h*RPC + jl] = pb[:, kh*128 + jl*4 + h]
            nc.vector.tensor_copy(
                st[:, kh * H * RPC:(kh + 1) * H * RPC].rearrange(
                    "p (h j) -> p h j", h=H),
                pb[:, kh * H * RPC:(kh + 1) * H * RPC].rearrange(
                    "p (j h) -> p h j", h=H),
            )
        bshard = dram.tile([128, 2 * H * RPC], bf16)
        bgather = dram.tile([NCORES, 128, 2 * H * RPC], bf16)
        nc.gpsimd.dma_start(out=bshard[:], in_=st)
        nc.gpsimd.collective_compute(
            "AllGather", mybir.AluOpType.bypass,
            replica_groups=[list(range(NCORES))],
            ins=[bshard.opt()], outs=[bgather.opt()])

        # --- projections + v for ALL rows (overlap the collective)
        for wsb, dst in ((wq_sb, qTall), (wk_sb, kTall)):
            for c in range(RPC * N // 512):
                pp = m_psum.tile([128, 512], f32, tag="m")
                nc.tensor.matmul(
                    pp, wsb, xt_sb[:, c * 512:(c + 1) * 512],
                    start=True, stop=True)
                nc.vector.tensor_copy(dst[:, c * 512:(c + 1) * 512], pp)
        for r in range(RPC):
            roff = r * N
            pv = m_psum.tile([128, 512], f32, tag="m")
            for half in range(2):
                nc.tensor.matmul(
                    pv[:, half * 128:(half + 1) * 128],
                    xt_sb[:, roff + half * 128: roff + half * 128 + 128],
                    wv_sb,
                    start=True, stop=True)
            nc.vector.tensor_copy(vall[:, roff: roff + N], pv[:, 0:N])

        bias_v = bias_sb.rearrange("p (kh h j) -> p kh h j", kh=2, h=H)
        for c in range(NCORES):
            # bias_v[:, kh, h, c*RPC + jl] = bgather[c, :, kh*128 + h*RPC + jl]
            nc.sync.dma_start(
                out=bias_v[:, :, :, c * RPC:(c + 1) * RPC],
                in_=bgather[c].rearrange("p (kh h j) -> p kh h j", kh=2, h=H))

        for r in range(RPC):
            roff = r * N
            est = {}
            for kh in range(2):
                for hp in range(2):
                    sp = s_psum.tile([128, 512], f32, tag="s")
                    for hl in range(2):
                        h = 2 * hp + hl
                        nc.tensor.matmul(
                            sp[:, hl * N:(hl + 1) * N],
                            ident,
                            bias_sb[:, kh * H * N + h * N: kh * H * N + (h + 1) * N],
                            start=True, stop=False)
                        nc.tensor.matmul(
                            sp[:, hl * N:(hl + 1) * N],
                            kTall[32 * h:32 * h + 32, roff + kh * 128: roff + kh * 128 + 128],
                            qTall[32 * h:32 * h + 32, roff: roff + N],
                            start=False, stop=True,
                            tile_position=(32 * h, 0))
                    es = es_pool.tile([128, 512], bf16, tag="es")
                    nc.scalar.activation(es, sp, AF.Exp)
                    est[(kh, hp)] = es

            # --- rowsums (replicated x32 via ones[128,E]) and AV
            po = o_psum.tile([128, 512], f32, tag="o")
            for h in range(H):
                for kh in range(2):
                    nc.tensor.matmul(
                        po[32 * h:32 * h + 32, 256:512],
                        ones,
                        est[(kh, h // 2)][:, (h % 2) * N:(h % 2 + 1) * N],
                        start=(kh == 0), stop=(kh == 1),
                        tile_position=(0, 32 * h))
            for h in range(H):
                for kh in range(2):
                    nc.tensor.matmul(
                        po[32 * h:32 * h + 32, 0:256],
                        vall[:, roff + kh * 128 + 32 * h: roff + kh * 128 + 32 * h + 32],
                        est[(kh, h // 2)][:, (h % 2) * N:(h % 2 + 1) * N],
                        start=(kh == 0), stop=(kh == 1),
                        tile_position=(0, 32 * h))

            rs_rec = sm_pool.tile([128, N], f32, tag="rs")
            nc.vector.reciprocal(rs_rec, po[:, 256:512])
            oT_sb = sm_pool.tile([128, N], bf16, tag="oT")
            nc.vector.tensor_mul(oT_sb, po[:, 0:256], rs_rec)

            # --- output projection: outT[d, j] = sum_he Wo[he,d] oT[he,j]
            pf = m_psum.tile([128, 512], f32, tag="m")
            nc.tensor.matmul(pf[:, 0:N], wo_sb, oT_sb, start=True, stop=True)
            o_sb = out_pool.tile([128, N], f32, tag="osb")
            nc.vector.tensor_copy(o_sb, pf[:, 0:N])
            nc.sync.dma_start(out=outT[r], in_=o_sb)
    return nc


def _get_programs():
    if "nc" not in _cache:
        _cache["nc"] = _build_fused()
        _legalize_waits(_cache["nc"])
    return _cache["nc"]


def kernel(pairwise_repr, mask, Wb, Wq, Wk, Wv, Wo):
    import ml_dtypes
    from concourse.bass_utils import run_bass_kernel_spmd

    bf = ml_dtypes.bfloat16
    nc = _get_programs()

    x = np.asarray(pairwise_repr, dtype=np.float32)[0]
    # xT[d, i*N + t] = x[i, t, d]
    xT = np.ascontiguousarray(x.reshape(N * N, D).T.astype(bf))
    shards = [np.ascontiguousarray(xT[:, c * RPC * N:(c + 1) * RPC * N])
              for c in range(NCORES)]
    wb = np.asarray(Wb, np.float32).astype(bf)
    # fold the attention scale into Wq so q comes out of the PE pre-scaled
    wq = (np.asarray(Wq, np.float32) * SCALE).astype(bf)
    wk = np.asarray(Wk, np.float32).astype(bf)
    wv = np.asarray(Wv, np.float32).astype(bf)
    wo = np.asarray(Wo, np.float32).astype(bf)

    core_ids = list(range(NCORES))
    in_maps = [{"xT": shards[c], "wb": wb, "wq": wq, "wk": wk,
                "wv": wv, "wo": wo} for c in range(NCORES)]
    kernel._last_in = in_maps
    res = run_bass_kernel_spmd(nc, in_maps, core_ids=core_ids, trace=False)

    kernel._last = res
    # outT [RPC, D, N] per core -> out[0, 32c+r, j, d] = outT_c[r, d, j]
    o = np.stack([res.results[c]["outT"] for c in range(NCORES)])
    out = o.transpose(0, 1, 3, 2).reshape(1, N, N, D)
    return np.ascontiguousarray(out.astype(np.float32))
